# revision 20
# baseline (speedup 1.0000x reference)
"""Multi-head attention (B=2, S=4096, D=512, H=8) on 8 trn2 NeuronCores.

Sharding: query-row data-parallel. Core c handles batch c//4, query rows
(c%4)*1024:(c%4+1)*1024 — all 8 heads. Each core:
  phase 1: PE-transposes its batch's X, projects K^T [D,S] and V [S,D]
           (full sequence, replicated across the 4 cores of a batch).
  phase 2: per 256-wide query chunk: project Q^T, then flash-style
           attention fully on-chip in transposed layout:
           S^T[k,q] = (K^T slice).T @ (Q^T slice) on PE, optional additive
           mask on DVE, exp on ACT, P^T @ V_aug on PE where V_aug carries
           a ones column so the softmax denominator falls out of the same
           matmul; per-head normalization via reciprocal + rank-1 PE
           broadcast.
  phase 3: output projection with per-head [64,q] lhsT chunks + bias.
No collectives; the host slices inputs per core and concatenates outputs.
Matmuls run as float32r (full-rate fp32) with fp32 PSUM accumulation.
"""

import numpy as np

# Problem dims (hardcoded per contract)
B, S, D, H, PD = 2, 4096, 512, 8, 64
P = 128
NCORES = 8
CPB = 4           # cores per batch
QR = S // CPB     # 1024 query rows per core
QC = 256          # query-chunk width in attention phase
NQC = QR // QC    # 4
SC = 256          # sequence chunk in KV-projection phase
NSC = S // SC     # 16
NKT = S // P      # 32 key tiles of 128
DC = D // P       # 4 d-chunks of 128
HP = H // 2       # 4 head-pairs
HG = 4            # heads per attention group (PSUM budget)
NHG = H // HG     # 2

_prog_cache = {}


def _build(masked: bool):
    import concourse.mybir as mybir
    import concourse.tile as tile
    from concourse import bacc
    from concourse.masks import make_identity

    f32 = mybir.dt.float32
    f32r = mybir.dt.float32r
    Exp = mybir.ActivationFunctionType.Exp
    Alu = mybir.AluOpType

    nc = bacc.Bacc(debug=False, target_bir_lowering=False)

    xb = nc.declare_dram_parameter("xb", [S, D], f32, isOutput=False)
    xq = nc.declare_dram_parameter("xq", [QR, D], f32, isOutput=False)
    wq_d = nc.declare_dram_parameter("wq", [P, DC, D], f32r, isOutput=False)
    wk_d = nc.declare_dram_parameter("wk", [P, DC, D], f32r, isOutput=False)
    wv_d = nc.declare_dram_parameter("wv", [P, DC, D], f32r, isOutput=False)
    wo_d = nc.declare_dram_parameter("wo", [P, DC, D], f32r, isOutput=False)
    bq_d = nc.declare_dram_parameter("bq", [P, DC], f32, isOutput=False)
    bk_d = nc.declare_dram_parameter("bk", [P, DC], f32, isOutput=False)
    bv_d = nc.declare_dram_parameter("bv", [P, D], f32, isOutput=False)
    bo_d = nc.declare_dram_parameter("bo", [P, D], f32, isOutput=False)
    ones_d = nc.declare_dram_parameter("ones", [P, H], f32r, isOutput=False)
    onesr_d = nc.declare_dram_parameter("onesr", [1, PD], f32r, isOutput=False)
    if masked:
        maskT_d = nc.declare_dram_parameter("maskT", [S, QR], f32, isOutput=False)
    out_d = nc.declare_dram_parameter("out", [QR, D], f32, isOutput=True)

    with tile.TileContext(nc) as tc, nc.allow_low_precision(
            reason="float32r tiles are 4-byte fp32; PE rounds reads only"):
        with (
            tc.tile_pool(name="const", bufs=1) as constp,
            tc.tile_pool(name="kt", bufs=1) as ktp,
            tc.tile_pool(name="vt", bufs=1) as vtp,
            tc.tile_pool(name="work", bufs=2) as work,
        ):
            ident = constp.tile([P, P], f32, tag="ident")
            make_identity(nc, ident)
            ones_col = constp.tile([1, PD], f32r, tag="ones")
            nc.sync.dma_start(ones_col[:], onesr_d[:])

            # weights/biases needed beyond phase 1
            wq = constp.tile([P, DC, D], f32r, tag="wq")
            wo = constp.tile([P, DC, D], f32r, tag="wo")
            bq = constp.tile([P, DC], f32, tag="bq")
            bo = constp.tile([P, D], f32, tag="bo")
            for sb_t, dr_t in [(wq, wq_d), (wo, wo_d), (bq, bq_d), (bo, bo_d)]:
                nc.sync.dma_start(sb_t[:], dr_t[:])

            # K^T tiles: kts[sci][p, hp, j] = K^T[hp*128+p, sci*SC+j]
            kts = [ktp.tile([P, HP, SC], f32r, tag=f"kt{i}", name=f"kt{i}")
                   for i in range(NSC)]
            # V tiles: vts[kti][p, h, 0:64] = V[kti*128+p, h*64:(h+1)*64],
            # col 64 = 1.0 (softmax-denominator row of the PV matmul)
            vts = [vtp.tile([P, H, PD + 1], f32r, tag=f"v{i}", name=f"v{i}")
                   for i in range(NKT)]
            for t in vts:
                nc.sync.dma_start(t[:, :, PD:PD + 1], ones_d[:, :, None])

            # ---- phase 1: transpose X chunks, project K^T and V ----
            with (
                tc.tile_pool(name="p1w", bufs=1) as p1w,
                tc.tile_pool(name="ps1", bufs=2, space="PSUM") as ps1,
            ):
                wk = p1w.tile([P, DC, D], f32r, tag="wk")
                wv = p1w.tile([P, DC, D], f32r, tag="wv")
                bk = p1w.tile([P, DC], f32, tag="bk")
                bv = p1w.tile([P, D], f32, tag="bv")
                for sb_t, dr_t in [(wk, wk_d), (wv, wv_d), (bk, bk_d), (bv, bv_d)]:
                    nc.sync.dma_start(sb_t[:], dr_t[:])

                for sci in range(NSC):
                    xraw = work.tile([P, SC // P, D], f32, tag="xraw")
                    nc.sync.dma_start(
                        xraw[:],
                        xb[sci * SC:(sci + 1) * SC].rearrange(
                            "(rt p) d -> p rt d", p=P),
                    )
                    xt = work.tile([P, DC, SC], f32r, tag="xt")
                    for rt in range(SC // P):
                        pst = ps1.tile([P, D], f32, tag="tps")
                        for dc in range(DC):
                            nc.tensor.transpose(
                                pst[:, dc * P:(dc + 1) * P],
                                xraw[:, rt, dc * P:(dc + 1) * P],
                                ident[:],
                            )
                        # pst[p, dc*128+j] = X^T[dc*128+p, sci*SC+rt*128+j]
                        nc.scalar.copy(
                            out=xt[:, :, rt * P:(rt + 1) * P],
                            in_=pst[:].rearrange("p (dc j) -> p dc j", dc=DC),
                        )
                    # K^T for this chunk
                    for hp in range(HP):
                        psk = ps1.tile([P, SC], f32, tag="kproj")
                        for dc in range(DC):
                            nc.tensor.matmul(
                                psk[:],
                                wk[:, dc, hp * P:(hp + 1) * P],
                                xt[:, dc, :],
                                start=(dc == 0), stop=(dc == DC - 1),
                            )
                        nc.scalar.add(kts[sci][:, hp, :], psk[:], bk[:, hp:hp + 1])
                    # V for this chunk
                    for rt in range(SC // P):
                        psv = ps1.tile([P, D], f32, tag="vproj")
                        for dc in range(DC):
                            nc.tensor.matmul(
                                psv[:],
                                xt[:, dc, rt * P:(rt + 1) * P],
                                wv[:, dc, :],
                                start=(dc == 0), stop=(dc == DC - 1),
                            )
                        kti = sci * (SC // P) + rt
                        nc.vector.tensor_add(
                            out=vts[kti][:, :, 0:PD],
                            in0=psv[:].rearrange("p (h d) -> p h d", h=H),
                            in1=bv[:].rearrange("p (h d) -> p h d", h=H),
                        )

            # ---- phase 2+3: attention + output projection per q-chunk ----
            with (
                tc.tile_pool(name="p2", bufs=2) as p2,
                tc.tile_pool(name="p2s", bufs=3) as p2s,
                tc.tile_pool(name="p2a", bufs=1) as p2a,
                tc.tile_pool(name="qkps", bufs=2, space="PSUM") as qkps,
                tc.tile_pool(name="pvps", bufs=1, space="PSUM") as pvps,
                tc.tile_pool(name="bcps", bufs=1, space="PSUM") as bcps,
                tc.tile_pool(name="fps", bufs=1, space="PSUM") as fps,
            ):
                for qc in range(NQC):
                    # Q^T for this chunk: qt[p, hp, j] = Q^T[hp*128+p, qc*QC+j]
                    xqraw = work.tile([P, QC // P, D], f32, tag="xraw")
                    nc.sync.dma_start(
                        xqraw[:],
                        xq[qc * QC:(qc + 1) * QC].rearrange(
                            "(rt p) d -> p rt d", p=P),
                    )
                    xqt = work.tile([P, DC, QC], f32r, tag="xt")
                    for rt in range(QC // P):
                        pst = qkps.tile([P, D], f32, tag="qk")
                        for dc in range(DC):
                            nc.tensor.transpose(
                                pst[:, dc * P:(dc + 1) * P],
                                xqraw[:, rt, dc * P:(dc + 1) * P],
                                ident[:],
                            )
                        nc.scalar.copy(
                            out=xqt[:, :, rt * P:(rt + 1) * P],
                            in_=pst[:].rearrange("p (dc j) -> p dc j", dc=DC),
                        )
                    qt = p2.tile([P, HP, QC], f32r, tag="qt")
                    for hp in range(HP):
                        psq = qkps.tile([P, D], f32, tag="qk")
                        for dc in range(DC):
                            nc.tensor.matmul(
                                psq[:, 0:QC],
                                wq[:, dc, hp * P:(hp + 1) * P],
                                xqt[:, dc, :],
                                start=(dc == 0), stop=(dc == DC - 1),
                            )
                        nc.scalar.add(qt[:, hp, :], psq[:, 0:QC], bq[:, hp:hp + 1])

                    # attnT[p, dc, j] = attn^T[dc*128+p, qc*QC+j]
                    attnT = p2a.tile([P, DC, QC], f32r, tag="attnT")
                    for hg in range(NHG):
                        heads = range(hg * HG, (hg + 1) * HG)
                        pvs = {h: pvps.tile([PD + 1, QC], f32, tag=f"pv{h % HG}",
                                            name=f"pv{h}")
                               for h in heads}
                        for kc in range(NKT):
                            if masked:
                                mt = p2s.tile([P, QC], f32, tag="mt")
                                nc.sync.dma_start(
                                    mt[:],
                                    maskT_d[kc * P:(kc + 1) * P,
                                            qc * QC:(qc + 1) * QC],
                                )
                            for h in heads:
                                po = (h % 2) * PD
                                pss = qkps.tile([P, D], f32, tag="qk")
                                nc.tensor.matmul(
                                    pss[:, 0:QC],
                                    kts[kc // 2][po:po + PD, h // 2,
                                                 (kc % 2) * P:(kc % 2 + 1) * P],
                                    qt[po:po + PD, h // 2, :],
                                    start=True, stop=True,
                                )
                                pt = p2s.tile([P, QC], f32r, tag="pt")
                                if masked:
                                    st = p2s.tile([P, QC], f32, tag="st")
                                    nc.vector.scalar_tensor_tensor(
                                        out=st[:], in0=mt[:], scalar=-1e9,
                                        in1=pss[:, 0:QC],
                                        op0=Alu.mult, op1=Alu.add,
                                    )
                                    nc.scalar.activation(pt[:], st[:], Exp,
                                                         scale=0.125)
                                else:
                                    nc.scalar.activation(pt[:], pss[:, 0:QC], Exp,
                                                         scale=0.125)
                                nc.tensor.matmul(
                                    pvs[h][:],
                                    vts[kc][:, h, :],
                                    pt[:],
                                    start=(kc == 0), stop=(kc == NKT - 1),
                                    skip_group_check=True,
                                )
                        for h in heads:
                            recip = p2s.tile([1, QC], f32r, tag="recip")
                            nc.vector.reciprocal(recip[:], pvs[h][PD:PD + 1, :])
                            bcp = bcps.tile([PD, QC], f32, tag="bc")
                            nc.tensor.matmul(
                                bcp[:], ones_col[:], recip[:],
                                start=True, stop=True,
                            )
                            bcs = p2s.tile([PD, QC], f32, tag="bcs")
                            nc.vector.tensor_copy(out=bcs[:], in_=bcp[:])
                            po = (h % 2) * PD
                            nc.vector.tensor_mul(
                                out=attnT[po:po + PD, h // 2, :],
                                in0=pvs[h][0:PD, :],
                                in1=bcs[:],
                            )

                    # output projection for this q-chunk
                    for rt in range(QC // P):
                        psf = fps.tile([P, D], f32, tag="fin")
                        for dc in range(DC):
                            nc.tensor.matmul(
                                psf[:],
                                attnT[:, dc, rt * P:(rt + 1) * P],
                                wo[:, dc, :],
                                start=(dc == 0), stop=(dc == DC - 1),
                            )
                        osb = p2s.tile([P, D], f32, tag="osb")
                        nc.vector.tensor_add(out=osb[:], in0=psf[:], in1=bo[:])
                        nc.sync.dma_start(
                            out_d[qc * QC + rt * P: qc * QC + (rt + 1) * P, :],
                            osb[:],
                        )
    nc.finalize()
    return nc


def _get_prog(masked: bool):
    if masked not in _prog_cache:
        _prog_cache[masked] = _build(masked)
    return _prog_cache[masked]


def _prep_shared(Wq, bq, Wk, bk, Wv, bv, Wo, bo):
    def warr(W):
        return np.ascontiguousarray(
            np.asarray(W, dtype=np.float32).reshape(DC, P, D).transpose(1, 0, 2))

    def barr(b):
        return np.ascontiguousarray(
            np.asarray(b, dtype=np.float32).reshape(DC, P).T)

    return {
        "wq": warr(Wq), "wk": warr(Wk), "wv": warr(Wv),
        "wo": warr(Wo),
        "bq": barr(bq), "bk": barr(bk),
        "bv": np.ascontiguousarray(
            np.broadcast_to(np.asarray(bv, dtype=np.float32), (P, D))),
        "bo": np.ascontiguousarray(
            np.broadcast_to(np.asarray(bo, dtype=np.float32), (P, D))),
    }


def make_in_maps(inputs, mask, Wq, bq, Wk, bk, Wv, bv, Wo, bo):
    inputs = np.asarray(inputs, dtype=np.float32)
    mask = np.asarray(mask, dtype=np.float32)
    masked = bool(np.any(mask))
    shared = _prep_shared(Wq, bq, Wk, bk, Wv, bv, Wo, bo)
    maskT = np.ascontiguousarray(mask.T) if masked else None

    in_maps = []
    for c in range(NCORES):
        b = c // CPB
        r0 = (c % CPB) * QR
        m = dict(shared)
        m["ones"] = np.ones((P, H), dtype=np.float32)
        m["onesr"] = np.ones((1, PD), dtype=np.float32)
        m["xb"] = np.ascontiguousarray(inputs[b])
        m["xq"] = np.ascontiguousarray(inputs[b, r0:r0 + QR])
        if masked:
            m["maskT"] = np.ascontiguousarray(maskT[:, r0:r0 + QR])
        in_maps.append(m)
    return masked, in_maps


def assemble(results):
    out = np.empty((B, S, D), dtype=np.float32)
    for c in range(NCORES):
        b = c // CPB
        r0 = (c % CPB) * QR
        out[b, r0:r0 + QR] = results[c]["out"]
    return out


def kernel(inputs, mask, Wq, bq, Wk, bk, Wv, bv, Wo, bo):
    from concourse.bass_utils import run_bass_kernel_spmd

    masked, in_maps = make_in_maps(inputs, mask, Wq, bq, Wk, bk, Wv, bv, Wo, bo)
    nc = _get_prog(masked)
    res = run_bass_kernel_spmd(nc, in_maps, core_ids=list(range(NCORES)))
    return assemble(res.results)


# revision 21
# speedup vs baseline: 1.2308x; 1.2308x over previous
"""Multi-head attention (B=2, S=4096, D=512, H=8) on 8 trn2 NeuronCores.

Sharding: query-row data-parallel. Core c handles batch c//4, query rows
(c%4)*1024:(c%4+1)*1024 — all 8 heads. Each core:
  phase 1: PE-transposes its batch's X, projects K^T [D,S] and V [S,D]
           (full sequence, replicated across the 4 cores of a batch).
  phase 2: per 256-wide query chunk: project Q^T, then flash-style
           attention fully on-chip in transposed layout:
           S^T[k,q] = (K^T slice).T @ (Q^T slice) on PE, optional additive
           mask on DVE, exp on ACT, P^T @ V_aug on PE where V_aug carries
           a ones column so the softmax denominator falls out of the same
           matmul; per-head normalization via reciprocal + rank-1 PE
           broadcast.
  phase 3: output projection with per-head [64,q] lhsT chunks + bias.
No collectives; the host slices inputs per core and concatenates outputs.
Matmuls run as float32r (full-rate fp32) with fp32 PSUM accumulation.
"""

import numpy as np

# Problem dims (hardcoded per contract)
B, S, D, H, PD = 2, 4096, 512, 8, 64
P = 128
NCORES = 8
CPB = 4           # cores per batch
QR = S // CPB     # 1024 query rows per core
QC = 256          # query-chunk width in attention phase
NQC = QR // QC    # 4
SC = 256          # sequence chunk in KV-projection phase
NSC = S // SC     # 16
NKT = S // P      # 32 key tiles of 128
DC = D // P       # 4 d-chunks of 128
HP = H // 2       # 4 head-pairs
HG = 4            # heads per attention group (PSUM budget)
NHG = H // HG     # 2

_prog_cache = {}


def _build(masked: bool):
    import concourse.mybir as mybir
    import concourse.tile as tile
    from concourse import bacc
    from concourse.masks import make_identity

    f32 = mybir.dt.float32
    f32r = mybir.dt.float32r
    bf16 = mybir.dt.bfloat16
    Exp = mybir.ActivationFunctionType.Exp
    Alu = mybir.AluOpType

    nc = bacc.Bacc(debug=False, target_bir_lowering=False)

    xb = nc.declare_dram_parameter("xb", [S, D], f32, isOutput=False)
    xq = nc.declare_dram_parameter("xq", [QR, D], f32, isOutput=False)
    wq_d = nc.declare_dram_parameter("wq", [P, DC, D], f32r, isOutput=False)
    wk_d = nc.declare_dram_parameter("wk", [P, DC, D], f32r, isOutput=False)
    wv_d = nc.declare_dram_parameter("wv", [P, DC, D], f32r, isOutput=False)
    wo_d = nc.declare_dram_parameter("wo", [P, DC, D], f32r, isOutput=False)
    bq_d = nc.declare_dram_parameter("bq", [P, DC], f32, isOutput=False)
    bk_d = nc.declare_dram_parameter("bk", [P, DC], f32, isOutput=False)
    bv_d = nc.declare_dram_parameter("bv", [P, D], f32, isOutput=False)
    bo_d = nc.declare_dram_parameter("bo", [P, D], f32, isOutput=False)
    ones_d = nc.declare_dram_parameter("ones", [P, H], bf16, isOutput=False)
    onesr_d = nc.declare_dram_parameter("onesr", [1, PD], f32r, isOutput=False)
    if masked:
        maskT_d = nc.declare_dram_parameter("maskT", [S, QR], f32, isOutput=False)
    out_d = nc.declare_dram_parameter("out", [QR, D], f32, isOutput=True)

    with tile.TileContext(nc) as tc, nc.allow_low_precision(
            reason="float32r tiles are 4-byte fp32; PE rounds reads only"):
        with (
            tc.tile_pool(name="const", bufs=1) as constp,
            tc.tile_pool(name="kt", bufs=1) as ktp,
            tc.tile_pool(name="vt", bufs=1) as vtp,
            tc.tile_pool(name="work", bufs=2) as work,
        ):
            ident = constp.tile([P, P], f32, tag="ident")
            make_identity(nc, ident)
            ones_col = constp.tile([1, PD], f32r, tag="ones")
            nc.sync.dma_start(ones_col[:], onesr_d[:])

            # weights/biases needed beyond phase 1
            wq = constp.tile([P, DC, D], f32r, tag="wq")
            wo = constp.tile([P, DC, D], f32r, tag="wo")
            bq = constp.tile([P, DC], f32, tag="bq")
            bo = constp.tile([P, D], f32, tag="bo")
            for sb_t, dr_t in [(wq, wq_d), (wo, wo_d), (bq, bq_d), (bo, bo_d)]:
                nc.sync.dma_start(sb_t[:], dr_t[:])

            # K^T tiles: kts[sci][p, hp, j] = K^T[hp*128+p, sci*SC+j]
            kts = [ktp.tile([P, HP, SC], bf16, tag=f"kt{i}", name=f"kt{i}")
                   for i in range(NSC)]
            # V tiles: vts[kti][p, h, 0:64] = V[kti*128+p, h*64:(h+1)*64],
            # col 64 = 1.0 (softmax-denominator row of the PV matmul)
            vts = [vtp.tile([P, H, PD + 1], bf16, tag=f"v{i}", name=f"v{i}")
                   for i in range(NKT)]
            for t in vts:
                nc.sync.dma_start(t[:, :, PD:PD + 1], ones_d[:, :, None])

            # ---- phase 1: transpose X chunks, project K^T and V ----
            with (
                tc.tile_pool(name="p1w", bufs=1) as p1w,
                tc.tile_pool(name="ps1", bufs=2, space="PSUM") as ps1,
            ):
                wk = p1w.tile([P, DC, D], f32r, tag="wk")
                wv = p1w.tile([P, DC, D], f32r, tag="wv")
                bk = p1w.tile([P, DC], f32, tag="bk")
                bv = p1w.tile([P, D], f32, tag="bv")
                for sb_t, dr_t in [(wk, wk_d), (wv, wv_d), (bk, bk_d), (bv, bv_d)]:
                    nc.sync.dma_start(sb_t[:], dr_t[:])

                for sci in range(NSC):
                    xraw = work.tile([P, SC // P, D], f32, tag="xraw")
                    nc.sync.dma_start(
                        xraw[:],
                        xb[sci * SC:(sci + 1) * SC].rearrange(
                            "(rt p) d -> p rt d", p=P),
                    )
                    xt = work.tile([P, DC, SC], f32r, tag="xt")
                    for rt in range(SC // P):
                        pst = ps1.tile([P, D], f32, tag="tps")
                        for dc in range(DC):
                            nc.tensor.transpose(
                                pst[:, dc * P:(dc + 1) * P],
                                xraw[:, rt, dc * P:(dc + 1) * P],
                                ident[:],
                            )
                        # pst[p, dc*128+j] = X^T[dc*128+p, sci*SC+rt*128+j]
                        nc.scalar.copy(
                            out=xt[:, :, rt * P:(rt + 1) * P],
                            in_=pst[:].rearrange("p (dc j) -> p dc j", dc=DC),
                        )
                    # K^T for this chunk
                    for hp in range(HP):
                        psk = ps1.tile([P, SC], f32, tag="kproj")
                        for dc in range(DC):
                            nc.tensor.matmul(
                                psk[:],
                                wk[:, dc, hp * P:(hp + 1) * P],
                                xt[:, dc, :],
                                start=(dc == 0), stop=(dc == DC - 1),
                            )
                        nc.scalar.add(kts[sci][:, hp, :], psk[:], bk[:, hp:hp + 1])
                    # V for this chunk
                    for rt in range(SC // P):
                        psv = ps1.tile([P, D], f32, tag="vproj")
                        for dc in range(DC):
                            nc.tensor.matmul(
                                psv[:],
                                xt[:, dc, rt * P:(rt + 1) * P],
                                wv[:, dc, :],
                                start=(dc == 0), stop=(dc == DC - 1),
                            )
                        kti = sci * (SC // P) + rt
                        nc.vector.tensor_add(
                            out=vts[kti][:, :, 0:PD],
                            in0=psv[:].rearrange("p (h d) -> p h d", h=H),
                            in1=bv[:].rearrange("p (h d) -> p h d", h=H),
                        )

            # ---- phase 2+3: attention + output projection per q-chunk ----
            with (
                tc.tile_pool(name="p2", bufs=2) as p2,
                tc.tile_pool(name="p2s", bufs=3) as p2s,
                tc.tile_pool(name="p2a", bufs=1) as p2a,
                tc.tile_pool(name="qkps", bufs=2, space="PSUM") as qkps,
                tc.tile_pool(name="pvps", bufs=1, space="PSUM") as pvps,
                tc.tile_pool(name="bcps", bufs=1, space="PSUM") as bcps,
                tc.tile_pool(name="fps", bufs=1, space="PSUM") as fps,
            ):
                for qc in range(NQC):
                    # Q^T for this chunk: qt[p, hp, j] = Q^T[hp*128+p, qc*QC+j]
                    xqraw = work.tile([P, QC // P, D], f32, tag="xraw")
                    nc.sync.dma_start(
                        xqraw[:],
                        xq[qc * QC:(qc + 1) * QC].rearrange(
                            "(rt p) d -> p rt d", p=P),
                    )
                    xqt = work.tile([P, DC, QC], f32r, tag="xt")
                    for rt in range(QC // P):
                        pst = qkps.tile([P, D], f32, tag="qk")
                        for dc in range(DC):
                            nc.tensor.transpose(
                                pst[:, dc * P:(dc + 1) * P],
                                xqraw[:, rt, dc * P:(dc + 1) * P],
                                ident[:],
                            )
                        nc.scalar.copy(
                            out=xqt[:, :, rt * P:(rt + 1) * P],
                            in_=pst[:].rearrange("p (dc j) -> p dc j", dc=DC),
                        )
                    qt = p2.tile([P, HP, QC], bf16, tag="qt")
                    for hp in range(HP):
                        psq = qkps.tile([P, D], f32, tag="qk")
                        for dc in range(DC):
                            nc.tensor.matmul(
                                psq[:, 0:QC],
                                wq[:, dc, hp * P:(hp + 1) * P],
                                xqt[:, dc, :],
                                start=(dc == 0), stop=(dc == DC - 1),
                            )
                        nc.scalar.add(qt[:, hp, :], psq[:, 0:QC], bq[:, hp:hp + 1])

                    # attnT[p, dc, j] = attn^T[dc*128+p, qc*QC+j]
                    attnT = p2a.tile([P, DC, QC], f32r, tag="attnT")
                    for hg in range(NHG):
                        heads = range(hg * HG, (hg + 1) * HG)
                        pvs = {h: pvps.tile([PD + 1, QC], f32, tag=f"pv{h % HG}",
                                            name=f"pv{h}")
                               for h in heads}
                        for kc in range(NKT):
                            if masked:
                                mt = p2s.tile([P, QC], f32, tag="mt")
                                nc.sync.dma_start(
                                    mt[:],
                                    maskT_d[kc * P:(kc + 1) * P,
                                            qc * QC:(qc + 1) * QC],
                                )
                            for h in heads:
                                po = (h % 2) * PD
                                pss = qkps.tile([P, D], f32, tag="qk")
                                nc.tensor.matmul(
                                    pss[:, 0:QC],
                                    kts[kc // 2][po:po + PD, h // 2,
                                                 (kc % 2) * P:(kc % 2 + 1) * P],
                                    qt[po:po + PD, h // 2, :],
                                    start=True, stop=True,
                                )
                                pt = p2s.tile([P, QC], bf16, tag="pt")
                                if masked:
                                    st = p2s.tile([P, QC], f32, tag="st")
                                    nc.vector.scalar_tensor_tensor(
                                        out=st[:], in0=mt[:], scalar=-1e9,
                                        in1=pss[:, 0:QC],
                                        op0=Alu.mult, op1=Alu.add,
                                    )
                                    nc.scalar.activation(pt[:], st[:], Exp,
                                                         scale=0.125)
                                else:
                                    nc.scalar.activation(pt[:], pss[:, 0:QC], Exp,
                                                         scale=0.125)
                                nc.tensor.matmul(
                                    pvs[h][:],
                                    vts[kc][:, h, :],
                                    pt[:],
                                    start=(kc == 0), stop=(kc == NKT - 1),
                                    skip_group_check=True,
                                )
                        for h in heads:
                            recip = p2s.tile([1, QC], f32r, tag="recip")
                            nc.vector.reciprocal(recip[:], pvs[h][PD:PD + 1, :])
                            bcp = bcps.tile([PD, QC], f32, tag="bc")
                            nc.tensor.matmul(
                                bcp[:], ones_col[:], recip[:],
                                start=True, stop=True,
                            )
                            bcs = p2s.tile([PD, QC], f32, tag="bcs")
                            nc.vector.tensor_copy(out=bcs[:], in_=bcp[:])
                            po = (h % 2) * PD
                            nc.vector.tensor_mul(
                                out=attnT[po:po + PD, h // 2, :],
                                in0=pvs[h][0:PD, :],
                                in1=bcs[:],
                            )

                    # output projection for this q-chunk
                    for rt in range(QC // P):
                        psf = fps.tile([P, D], f32, tag="fin")
                        for dc in range(DC):
                            nc.tensor.matmul(
                                psf[:],
                                attnT[:, dc, rt * P:(rt + 1) * P],
                                wo[:, dc, :],
                                start=(dc == 0), stop=(dc == DC - 1),
                            )
                        osb = p2s.tile([P, D], f32, tag="osb")
                        nc.vector.tensor_add(out=osb[:], in0=psf[:], in1=bo[:])
                        nc.sync.dma_start(
                            out_d[qc * QC + rt * P: qc * QC + (rt + 1) * P, :],
                            osb[:],
                        )
    nc.finalize()
    return nc


def _get_prog(masked: bool):
    if masked not in _prog_cache:
        _prog_cache[masked] = _build(masked)
    return _prog_cache[masked]


def _prep_shared(Wq, bq, Wk, bk, Wv, bv, Wo, bo):
    def warr(W):
        return np.ascontiguousarray(
            np.asarray(W, dtype=np.float32).reshape(DC, P, D).transpose(1, 0, 2))

    def barr(b):
        return np.ascontiguousarray(
            np.asarray(b, dtype=np.float32).reshape(DC, P).T)

    return {
        "wq": warr(Wq), "wk": warr(Wk), "wv": warr(Wv),
        "wo": warr(Wo),
        "bq": barr(bq), "bk": barr(bk),
        "bv": np.ascontiguousarray(
            np.broadcast_to(np.asarray(bv, dtype=np.float32), (P, D))),
        "bo": np.ascontiguousarray(
            np.broadcast_to(np.asarray(bo, dtype=np.float32), (P, D))),
    }


def make_in_maps(inputs, mask, Wq, bq, Wk, bk, Wv, bv, Wo, bo):
    inputs = np.asarray(inputs, dtype=np.float32)
    mask = np.asarray(mask, dtype=np.float32)
    masked = bool(np.any(mask))
    shared = _prep_shared(Wq, bq, Wk, bk, Wv, bv, Wo, bo)
    maskT = np.ascontiguousarray(mask.T) if masked else None

    in_maps = []
    for c in range(NCORES):
        b = c // CPB
        r0 = (c % CPB) * QR
        m = dict(shared)
        import ml_dtypes
        m["ones"] = np.ones((P, H), dtype=ml_dtypes.bfloat16)
        m["onesr"] = np.ones((1, PD), dtype=np.float32)
        m["xb"] = np.ascontiguousarray(inputs[b])
        m["xq"] = np.ascontiguousarray(inputs[b, r0:r0 + QR])
        if masked:
            m["maskT"] = np.ascontiguousarray(maskT[:, r0:r0 + QR])
        in_maps.append(m)
    return masked, in_maps


def assemble(results):
    out = np.empty((B, S, D), dtype=np.float32)
    for c in range(NCORES):
        b = c // CPB
        r0 = (c % CPB) * QR
        out[b, r0:r0 + QR] = results[c]["out"]
    return out


def kernel(inputs, mask, Wq, bq, Wk, bk, Wv, bv, Wo, bo):
    from concourse.bass_utils import run_bass_kernel_spmd

    masked, in_maps = make_in_maps(inputs, mask, Wq, bq, Wk, bk, Wv, bv, Wo, bo)
    nc = _get_prog(masked)
    res = run_bass_kernel_spmd(nc, in_maps, core_ids=list(range(NCORES)))
    return assemble(res.results)


# revision 23
# speedup vs baseline: 1.6675x; 1.3548x over previous
"""Multi-head attention (B=2, S=4096, D=512, H=8) on 8 trn2 NeuronCores.

Sharding: query-row data-parallel. Core c handles batch c//4, query rows
(c%4)*1024:(c%4+1)*1024 — all 8 heads. Each core:
  phase 1: PE-transposes its batch's X, projects K^T [D,S] and V [S,D]
           (full sequence, replicated across the 4 cores of a batch).
  phase 2: per 256-wide query chunk: project Q^T, then flash-style
           attention fully on-chip in transposed layout:
           S^T[k,q] = (K^T slice).T @ (Q^T slice) on PE, optional additive
           mask on DVE, exp on ACT, P^T @ V_aug on PE where V_aug carries
           a ones column so the softmax denominator falls out of the same
           matmul; per-head normalization via reciprocal + rank-1 PE
           broadcast.
  phase 3: output projection with per-head [64,q] lhsT chunks + bias.
No collectives; the host slices inputs per core and concatenates outputs.
Matmuls run as float32r (full-rate fp32) with fp32 PSUM accumulation.
"""

import numpy as np

# Problem dims (hardcoded per contract)
B, S, D, H, PD = 2, 4096, 512, 8, 64
P = 128
NCORES = 8
CPB = 4           # cores per batch
QR = S // CPB     # 1024 query rows per core
QC = 512          # query-chunk width in attention phase
NQC = QR // QC    # 2
SC = 512          # sequence chunk in KV-projection phase
NSC = S // SC     # 8
NKT = S // P      # 32 key tiles of 128
DC = D // P       # 4 d-chunks of 128
HP = H // 2       # 4 head-pairs
HG = 4            # heads per attention group (PSUM budget)
NHG = H // HG     # 2

_prog_cache = {}


def _build(mode: str):
    import concourse.mybir as mybir
    import concourse.tile as tile
    from concourse import bacc
    from concourse.masks import make_identity

    f32 = mybir.dt.float32
    f32r = mybir.dt.float32r
    bf16 = mybir.dt.bfloat16
    Exp = mybir.ActivationFunctionType.Exp
    Alu = mybir.AluOpType

    nc = bacc.Bacc(debug=False, target_bir_lowering=False)

    xb = nc.declare_dram_parameter("xb", [S, D], f32, isOutput=False)
    xq = nc.declare_dram_parameter("xq", [QR, D], f32, isOutput=False)
    wq_d = nc.declare_dram_parameter("wq", [P, DC, D], f32r, isOutput=False)
    wk_d = nc.declare_dram_parameter("wk", [P, DC, D], f32r, isOutput=False)
    wv_d = nc.declare_dram_parameter("wv", [P, DC, D], f32r, isOutput=False)
    wo_d = nc.declare_dram_parameter("wo", [P, DC, D], f32r, isOutput=False)
    bq_d = nc.declare_dram_parameter("bq", [P, DC], f32, isOutput=False)
    bk_d = nc.declare_dram_parameter("bk", [P, DC], f32, isOutput=False)
    bv_d = nc.declare_dram_parameter("bv", [P, D], f32, isOutput=False)
    bo_d = nc.declare_dram_parameter("bo", [P, D], f32, isOutput=False)
    ones_d = nc.declare_dram_parameter("ones", [P, H], bf16, isOutput=False)
    onesr_d = nc.declare_dram_parameter("onesr", [1, PD], f32r, isOutput=False)
    if mode == "add":
        maskT_d = nc.declare_dram_parameter("maskT", [S, QR], f32, isOutput=False)
    elif mode == "bin":
        maskT_d = nc.declare_dram_parameter("maskT", [S, QR], bf16, isOutput=False)
    out_d = nc.declare_dram_parameter("out", [QR, D], f32, isOutput=True)

    with tile.TileContext(nc) as tc, nc.allow_low_precision(
            reason="float32r tiles are 4-byte fp32; PE rounds reads only"):
        with (
            tc.tile_pool(name="const", bufs=1) as constp,
            tc.tile_pool(name="kt", bufs=1) as ktp,
            tc.tile_pool(name="vt", bufs=1) as vtp,
            tc.tile_pool(name="work", bufs=2) as work,
        ):
            ident = constp.tile([P, P], f32, tag="ident")
            make_identity(nc, ident)
            ones_col = constp.tile([1, PD], f32r, tag="ones")
            nc.sync.dma_start(ones_col[:], onesr_d[:])

            # weights/biases needed beyond phase 1
            wq = constp.tile([P, DC, D], f32r, tag="wq")
            wo = constp.tile([P, DC, D], f32r, tag="wo")
            bq = constp.tile([P, DC], f32, tag="bq")
            bo = constp.tile([P, D], f32, tag="bo")
            for sb_t, dr_t in [(wq, wq_d), (wo, wo_d), (bq, bq_d), (bo, bo_d)]:
                nc.sync.dma_start(sb_t[:], dr_t[:])

            # K^T tiles: kts[sci][p, hp, j] = K^T[hp*128+p, sci*SC+j]
            kts = [ktp.tile([P, HP, SC], bf16, tag=f"kt{i}", name=f"kt{i}")
                   for i in range(NSC)]
            # V tiles: vts[kti][p, h, 0:64] = V[kti*128+p, h*64:(h+1)*64],
            # col 64 = 1.0 (softmax-denominator row of the PV matmul)
            vts = [vtp.tile([P, H, PD + 1], bf16, tag=f"v{i}", name=f"v{i}")
                   for i in range(NKT)]
            for t in vts:
                nc.sync.dma_start(t[:, :, PD:PD + 1], ones_d[:, :, None])

            # ---- phase 1: transpose X chunks, project K^T and V ----
            with (
                tc.tile_pool(name="p1w", bufs=1) as p1w,
                tc.tile_pool(name="ps1", bufs=2, space="PSUM") as ps1,
            ):
                wk = p1w.tile([P, DC, D], f32r, tag="wk")
                wv = p1w.tile([P, DC, D], f32r, tag="wv")
                bk = p1w.tile([P, DC], f32, tag="bk")
                bv = p1w.tile([P, D], f32, tag="bv")
                for sb_t, dr_t in [(wk, wk_d), (wv, wv_d), (bk, bk_d), (bv, bv_d)]:
                    nc.sync.dma_start(sb_t[:], dr_t[:])

                for sci in range(NSC):
                    xraw = work.tile([P, SC // P, D], f32, tag="xraw")
                    nc.sync.dma_start(
                        xraw[:],
                        xb[sci * SC:(sci + 1) * SC].rearrange(
                            "(rt p) d -> p rt d", p=P),
                    )
                    xt = work.tile([P, DC, SC], f32r, tag="xt")
                    for rt in range(SC // P):
                        pst = ps1.tile([P, D], f32, tag="tps")
                        for dc in range(DC):
                            nc.tensor.transpose(
                                pst[:, dc * P:(dc + 1) * P],
                                xraw[:, rt, dc * P:(dc + 1) * P],
                                ident[:],
                            )
                        # pst[p, dc*128+j] = X^T[dc*128+p, sci*SC+rt*128+j]
                        nc.scalar.copy(
                            out=xt[:, :, rt * P:(rt + 1) * P],
                            in_=pst[:].rearrange("p (dc j) -> p dc j", dc=DC),
                        )
                    # K^T for this chunk
                    for hp in range(HP):
                        psk = ps1.tile([P, SC], f32, tag="kproj")
                        for dc in range(DC):
                            nc.tensor.matmul(
                                psk[:],
                                wk[:, dc, hp * P:(hp + 1) * P],
                                xt[:, dc, :],
                                start=(dc == 0), stop=(dc == DC - 1),
                            )
                        nc.scalar.add(kts[sci][:, hp, :], psk[:], bk[:, hp:hp + 1])
                    # V for this chunk
                    for rt in range(SC // P):
                        psv = ps1.tile([P, D], f32, tag="vproj")
                        for dc in range(DC):
                            nc.tensor.matmul(
                                psv[:],
                                xt[:, dc, rt * P:(rt + 1) * P],
                                wv[:, dc, :],
                                start=(dc == 0), stop=(dc == DC - 1),
                            )
                        kti = sci * (SC // P) + rt
                        nc.vector.tensor_add(
                            out=vts[kti][:, :, 0:PD],
                            in0=psv[:].rearrange("p (h d) -> p h d", h=H),
                            in1=bv[:].rearrange("p (h d) -> p h d", h=H),
                        )

            # ---- phase 2+3: attention + output projection per q-chunk ----
            with (
                tc.tile_pool(name="p2", bufs=2) as p2,
                tc.tile_pool(name="p2s", bufs=3) as p2s,
                tc.tile_pool(name="p2a", bufs=1) as p2a,
                tc.tile_pool(name="qkps", bufs=3, space="PSUM") as qkps,
                tc.tile_pool(name="pvps", bufs=1, space="PSUM") as pvps,
                tc.tile_pool(name="fps", bufs=1, space="PSUM") as fps,
            ):
                for qc in range(NQC):
                    # Q^T for this chunk: qt[p, hp, j] = Q^T[hp*128+p, qc*QC+j]
                    xqraw = work.tile([P, QC // P, D], f32, tag="xraw")
                    nc.sync.dma_start(
                        xqraw[:],
                        xq[qc * QC:(qc + 1) * QC].rearrange(
                            "(rt p) d -> p rt d", p=P),
                    )
                    xqt = work.tile([P, DC, QC], f32r, tag="xt")
                    for rt in range(QC // P):
                        pst = qkps.tile([P, D], f32, tag="qk")
                        for dc in range(DC):
                            nc.tensor.transpose(
                                pst[:, dc * P:(dc + 1) * P],
                                xqraw[:, rt, dc * P:(dc + 1) * P],
                                ident[:],
                            )
                        nc.scalar.copy(
                            out=xqt[:, :, rt * P:(rt + 1) * P],
                            in_=pst[:].rearrange("p (dc j) -> p dc j", dc=DC),
                        )
                    qt = p2.tile([P, HP, QC], bf16, tag="qt")
                    for hp in range(HP):
                        psq = qkps.tile([P, D], f32, tag="qk")
                        for dc in range(DC):
                            nc.tensor.matmul(
                                psq[:, 0:QC],
                                wq[:, dc, hp * P:(hp + 1) * P],
                                xqt[:, dc, :],
                                start=(dc == 0), stop=(dc == DC - 1),
                            )
                        nc.scalar.add(qt[:, hp, :], psq[:, 0:QC], bq[:, hp:hp + 1])

                    # attnT[p, dc, j] = attn^T[dc*128+p, qc*QC+j]
                    attnT = p2a.tile([P, DC, QC], f32r, tag="attnT")
                    for hg in range(NHG):
                        heads = range(hg * HG, (hg + 1) * HG)
                        pvs = {h: pvps.tile([PD + 1, QC], f32, tag=f"pv{h % HG}",
                                            name=f"pv{h}")
                               for h in heads}
                        for kc in range(NKT):
                            if mode == "add":
                                mt = p2s.tile([P, QC], f32, tag="mt")
                            elif mode == "bin":
                                mt = p2s.tile([P, QC], bf16, tag="mt")
                            if mode != "none":
                                nc.sync.dma_start(
                                    mt[:],
                                    maskT_d[kc * P:(kc + 1) * P,
                                            qc * QC:(qc + 1) * QC],
                                )
                            for h in heads:
                                po = (h % 2) * PD
                                pss = qkps.tile([P, D], f32, tag="qk")
                                nc.tensor.matmul(
                                    pss[:, 0:QC],
                                    kts[kc // (SC // P)][
                                        po:po + PD, h // 2,
                                        (kc % (SC // P)) * P:
                                        (kc % (SC // P) + 1) * P],
                                    qt[po:po + PD, h // 2, :],
                                    start=True, stop=True,
                                )
                                pt = p2s.tile([P, QC], bf16, tag="pt")
                                if mode == "add":
                                    st = p2s.tile([P, QC], f32, tag="st")
                                    nc.vector.scalar_tensor_tensor(
                                        out=st[:], in0=mt[:], scalar=-1e9,
                                        in1=pss[:, 0:QC],
                                        op0=Alu.mult, op1=Alu.add,
                                    )
                                    nc.scalar.activation(pt[:], st[:], Exp,
                                                         scale=0.125)
                                elif mode == "bin":
                                    pr = p2s.tile([P, QC], bf16, tag="pr")
                                    nc.scalar.activation(pr[:], pss[:, 0:QC], Exp,
                                                         scale=0.125)
                                    nc.vector.tensor_mul(
                                        out=pt[:], in0=pr[:], in1=mt[:])
                                else:
                                    nc.scalar.activation(pt[:], pss[:, 0:QC], Exp,
                                                         scale=0.125)
                                nc.tensor.matmul(
                                    pvs[h][:],
                                    vts[kc][:, h, :],
                                    pt[:],
                                    start=(kc == 0), stop=(kc == NKT - 1),
                                    skip_group_check=True,
                                )
                        for h in heads:
                            recip = p2s.tile([1, QC], f32r, tag="recip")
                            nc.vector.reciprocal(recip[:], pvs[h][PD:PD + 1, :])
                            bcp = fps.tile([PD, QC], f32, tag="fin")
                            nc.tensor.matmul(
                                bcp[:], ones_col[:], recip[:],
                                start=True, stop=True,
                            )
                            bcs = p2s.tile([PD, QC], f32, tag="bcs")
                            nc.vector.tensor_copy(out=bcs[:], in_=bcp[:])
                            po = (h % 2) * PD
                            nc.vector.tensor_mul(
                                out=attnT[po:po + PD, h // 2, :],
                                in0=pvs[h][0:PD, :],
                                in1=bcs[:],
                            )

                    # output projection for this q-chunk
                    for rt in range(QC // P):
                        psf = fps.tile([P, D], f32, tag="fin")
                        for dc in range(DC):
                            nc.tensor.matmul(
                                psf[:],
                                attnT[:, dc, rt * P:(rt + 1) * P],
                                wo[:, dc, :],
                                start=(dc == 0), stop=(dc == DC - 1),
                            )
                        osb = p2s.tile([P, D], f32, tag="osb")
                        nc.vector.tensor_add(out=osb[:], in0=psf[:], in1=bo[:])
                        nc.sync.dma_start(
                            out_d[qc * QC + rt * P: qc * QC + (rt + 1) * P, :],
                            osb[:],
                        )
    nc.finalize()
    return nc


def _get_prog(mode: str):
    if mode not in _prog_cache:
        _prog_cache[mode] = _build(mode)
    return _prog_cache[mode]


def _prep_shared(Wq, bq, Wk, bk, Wv, bv, Wo, bo):
    def warr(W):
        return np.ascontiguousarray(
            np.asarray(W, dtype=np.float32).reshape(DC, P, D).transpose(1, 0, 2))

    def barr(b):
        return np.ascontiguousarray(
            np.asarray(b, dtype=np.float32).reshape(DC, P).T)

    return {
        "wq": warr(Wq), "wk": warr(Wk), "wv": warr(Wv),
        "wo": warr(Wo),
        "bq": barr(bq), "bk": barr(bk),
        "bv": np.ascontiguousarray(
            np.broadcast_to(np.asarray(bv, dtype=np.float32), (P, D))),
        "bo": np.ascontiguousarray(
            np.broadcast_to(np.asarray(bo, dtype=np.float32), (P, D))),
    }


def make_in_maps(inputs, mask, Wq, bq, Wk, bk, Wv, bv, Wo, bo):
    import ml_dtypes
    inputs = np.asarray(inputs, dtype=np.float32)
    mask = np.asarray(mask, dtype=np.float32)
    if not np.any(mask):
        mode = "none"
        maskT = None
    elif bool(((mask == 0.0) | (mask == 1.0)).all()):
        mode = "bin"
        maskT = np.ascontiguousarray(1.0 - mask.T).astype(ml_dtypes.bfloat16)
    else:
        mode = "add"
        maskT = np.ascontiguousarray(mask.T)
    shared = _prep_shared(Wq, bq, Wk, bk, Wv, bv, Wo, bo)

    in_maps = []
    for c in range(NCORES):
        b = c // CPB
        r0 = (c % CPB) * QR
        m = dict(shared)
        import ml_dtypes
        m["ones"] = np.ones((P, H), dtype=ml_dtypes.bfloat16)
        m["onesr"] = np.ones((1, PD), dtype=np.float32)
        m["xb"] = np.ascontiguousarray(inputs[b])
        m["xq"] = np.ascontiguousarray(inputs[b, r0:r0 + QR])
        if maskT is not None:
            m["maskT"] = np.ascontiguousarray(maskT[:, r0:r0 + QR])
        in_maps.append(m)
    return mode, in_maps


def assemble(results):
    out = np.empty((B, S, D), dtype=np.float32)
    for c in range(NCORES):
        b = c // CPB
        r0 = (c % CPB) * QR
        out[b, r0:r0 + QR] = results[c]["out"]
    return out


def kernel(inputs, mask, Wq, bq, Wk, bk, Wv, bv, Wo, bo):
    from concourse.bass_utils import run_bass_kernel_spmd

    mode, in_maps = make_in_maps(inputs, mask, Wq, bq, Wk, bk, Wv, bv, Wo, bo)
    nc = _get_prog(mode)
    res = run_bass_kernel_spmd(nc, in_maps, core_ids=list(range(NCORES)))
    return assemble(res.results)


# revision 24
# speedup vs baseline: 2.0032x; 1.2013x over previous
"""Multi-head attention (B=2, S=4096, D=512, H=8) on 8 trn2 NeuronCores.

Sharding: query-row data-parallel. Core c handles batch c//4, query rows
(c%4)*1024:(c%4+1)*1024 — all 8 heads. Each core:
  phase 1: PE-transposes its batch's X, projects K^T [D,S] and V [S,D]
           (full sequence, replicated across the 4 cores of a batch).
  phase 2: per 256-wide query chunk: project Q^T, then flash-style
           attention fully on-chip in transposed layout:
           S^T[k,q] = (K^T slice).T @ (Q^T slice) on PE, optional additive
           mask on DVE, exp on ACT, P^T @ V_aug on PE where V_aug carries
           a ones column so the softmax denominator falls out of the same
           matmul; per-head normalization via reciprocal + rank-1 PE
           broadcast.
  phase 3: output projection with per-head [64,q] lhsT chunks + bias.
No collectives; the host slices inputs per core and concatenates outputs.
Matmuls run as float32r (full-rate fp32) with fp32 PSUM accumulation.
"""

import numpy as np

# Problem dims (hardcoded per contract)
B, S, D, H, PD = 2, 4096, 512, 8, 64
P = 128
NCORES = 8
CPB = 4           # cores per batch
QR = S // CPB     # 1024 query rows per core
QC = 512          # query-chunk width in attention phase
NQC = QR // QC    # 2
SC = 512          # sequence chunk in KV-projection phase
NSC = S // SC     # 8
NKT = S // P      # 32 key tiles of 128
DC = D // P       # 4 d-chunks of 128
HP = H // 2       # 4 head-pairs
HG = 4            # heads per attention group (PSUM budget)
NHG = H // HG     # 2

_prog_cache = {}


def _build(mode: str):
    import concourse.mybir as mybir
    import concourse.tile as tile
    from concourse import bacc
    from concourse.masks import make_identity

    f32 = mybir.dt.float32
    f32r = mybir.dt.float32r
    bf16 = mybir.dt.bfloat16
    Exp = mybir.ActivationFunctionType.Exp
    Alu = mybir.AluOpType

    nc = bacc.Bacc(debug=False, target_bir_lowering=False)

    xb = nc.declare_dram_parameter("xb", [S, D], f32, isOutput=False)
    xq = nc.declare_dram_parameter("xq", [QR, D], f32, isOutput=False)
    wq_d = nc.declare_dram_parameter("wq", [P, DC, D], f32r, isOutput=False)
    wk_d = nc.declare_dram_parameter("wk", [P, DC, D], f32r, isOutput=False)
    wv_d = nc.declare_dram_parameter("wv", [P, DC, D], f32r, isOutput=False)
    wo_d = nc.declare_dram_parameter("wo", [P, DC, D], f32r, isOutput=False)
    bq_d = nc.declare_dram_parameter("bq", [P, DC], f32, isOutput=False)
    bk_d = nc.declare_dram_parameter("bk", [P, DC], f32, isOutput=False)
    bv_d = nc.declare_dram_parameter("bv", [P, D], f32, isOutput=False)
    bo_d = nc.declare_dram_parameter("bo", [P, D], f32, isOutput=False)
    ones_d = nc.declare_dram_parameter("ones", [P, H], bf16, isOutput=False)
    onesr_d = nc.declare_dram_parameter("onesr", [1, PD], f32r, isOutput=False)
    if mode == "add":
        maskT_d = nc.declare_dram_parameter("maskT", [S, QR], f32, isOutput=False)
    elif mode in ("bin", "tril"):
        maskT_d = nc.declare_dram_parameter("maskT", [S, QR], bf16, isOutput=False)
    out_d = nc.declare_dram_parameter("out", [QR, D], f32, isOutput=True)

    with tile.TileContext(nc) as tc, nc.allow_low_precision(
            reason="float32r tiles are 4-byte fp32; PE rounds reads only"):
        with (
            tc.tile_pool(name="const", bufs=1) as constp,
            tc.tile_pool(name="kt", bufs=1) as ktp,
            tc.tile_pool(name="vt", bufs=1) as vtp,
            tc.tile_pool(name="work", bufs=2) as work,
        ):
            ident = constp.tile([P, P], f32, tag="ident")
            make_identity(nc, ident)
            ones_col = constp.tile([1, PD], f32r, tag="ones")
            nc.sync.dma_start(ones_col[:], onesr_d[:])

            # weights/biases needed beyond phase 1
            wq = constp.tile([P, DC, D], f32r, tag="wq")
            wo = constp.tile([P, DC, D], f32r, tag="wo")
            bq = constp.tile([P, DC], f32, tag="bq")
            bo = constp.tile([P, D], f32, tag="bo")
            for sb_t, dr_t in [(wq, wq_d), (wo, wo_d), (bq, bq_d), (bo, bo_d)]:
                nc.sync.dma_start(sb_t[:], dr_t[:])

            # K^T tiles: kts[sci][p, hp, j] = K^T[hp*128+p, sci*SC+j]
            kts = [ktp.tile([P, HP, SC], bf16, tag=f"kt{i}", name=f"kt{i}")
                   for i in range(NSC)]
            # V tiles: vts[kti][p, h, 0:64] = V[kti*128+p, h*64:(h+1)*64],
            # col 64 = 1.0 (softmax-denominator row of the PV matmul)
            vts = [vtp.tile([P, H, PD + 1], bf16, tag=f"v{i}", name=f"v{i}")
                   for i in range(NKT)]
            for t in vts:
                nc.sync.dma_start(t[:, :, PD:PD + 1], ones_d[:, :, None])

            # ---- phase 1: transpose X chunks, project K^T and V ----
            with (
                tc.tile_pool(name="p1w", bufs=1) as p1w,
                tc.tile_pool(name="ps1", bufs=2, space="PSUM") as ps1,
            ):
                wk = p1w.tile([P, DC, D], f32r, tag="wk")
                wv = p1w.tile([P, DC, D], f32r, tag="wv")
                bk = p1w.tile([P, DC], f32, tag="bk")
                bv = p1w.tile([P, D], f32, tag="bv")
                for sb_t, dr_t in [(wk, wk_d), (wv, wv_d), (bk, bk_d), (bv, bv_d)]:
                    nc.sync.dma_start(sb_t[:], dr_t[:])

                for sci in range(NSC):
                    xraw = work.tile([P, SC // P, D], f32, tag="xraw")
                    nc.sync.dma_start(
                        xraw[:],
                        xb[sci * SC:(sci + 1) * SC].rearrange(
                            "(rt p) d -> p rt d", p=P),
                    )
                    xt = work.tile([P, DC, SC], f32r, tag="xt")
                    for rt in range(SC // P):
                        pst = ps1.tile([P, D], f32, tag="tps")
                        for dc in range(DC):
                            nc.tensor.transpose(
                                pst[:, dc * P:(dc + 1) * P],
                                xraw[:, rt, dc * P:(dc + 1) * P],
                                ident[:],
                            )
                        # pst[p, dc*128+j] = X^T[dc*128+p, sci*SC+rt*128+j]
                        nc.scalar.copy(
                            out=xt[:, :, rt * P:(rt + 1) * P],
                            in_=pst[:].rearrange("p (dc j) -> p dc j", dc=DC),
                        )
                    # K^T for this chunk
                    for hp in range(HP):
                        psk = ps1.tile([P, SC], f32, tag="kproj")
                        for dc in range(DC):
                            nc.tensor.matmul(
                                psk[:],
                                wk[:, dc, hp * P:(hp + 1) * P],
                                xt[:, dc, :],
                                start=(dc == 0), stop=(dc == DC - 1),
                            )
                        nc.scalar.add(kts[sci][:, hp, :], psk[:], bk[:, hp:hp + 1])
                    # V for this chunk
                    for rt in range(SC // P):
                        psv = ps1.tile([P, D], f32, tag="vproj")
                        for dc in range(DC):
                            nc.tensor.matmul(
                                psv[:],
                                xt[:, dc, rt * P:(rt + 1) * P],
                                wv[:, dc, :],
                                start=(dc == 0), stop=(dc == DC - 1),
                            )
                        kti = sci * (SC // P) + rt
                        nc.vector.tensor_add(
                            out=vts[kti][:, :, 0:PD],
                            in0=psv[:].rearrange("p (h d) -> p h d", h=H),
                            in1=bv[:].rearrange("p (h d) -> p h d", h=H),
                        )

            # ---- phase 2+3: attention + output projection per q-chunk ----
            with (
                tc.tile_pool(name="p2", bufs=2) as p2,
                tc.tile_pool(name="p2s", bufs=3) as p2s,
                tc.tile_pool(name="p2a", bufs=1) as p2a,
                tc.tile_pool(name="qkps", bufs=3, space="PSUM") as qkps,
                tc.tile_pool(name="pvps", bufs=1, space="PSUM") as pvps,
                tc.tile_pool(name="fps", bufs=1, space="PSUM") as fps,
            ):
                for qc in range(NQC):
                    # Q^T for this chunk: qt[p, hp, j] = Q^T[hp*128+p, qc*QC+j]
                    xqraw = work.tile([P, QC // P, D], f32, tag="xraw")
                    nc.sync.dma_start(
                        xqraw[:],
                        xq[qc * QC:(qc + 1) * QC].rearrange(
                            "(rt p) d -> p rt d", p=P),
                    )
                    xqt = work.tile([P, DC, QC], f32r, tag="xt")
                    for rt in range(QC // P):
                        pst = qkps.tile([P, D], f32, tag="qk")
                        for dc in range(DC):
                            nc.tensor.transpose(
                                pst[:, dc * P:(dc + 1) * P],
                                xqraw[:, rt, dc * P:(dc + 1) * P],
                                ident[:],
                            )
                        nc.scalar.copy(
                            out=xqt[:, :, rt * P:(rt + 1) * P],
                            in_=pst[:].rearrange("p (dc j) -> p dc j", dc=DC),
                        )
                    qt = p2.tile([P, HP, QC], bf16, tag="qt")
                    for hp in range(HP):
                        psq = qkps.tile([P, D], f32, tag="qk")
                        for dc in range(DC):
                            nc.tensor.matmul(
                                psq[:, 0:QC],
                                wq[:, dc, hp * P:(hp + 1) * P],
                                xqt[:, dc, :],
                                start=(dc == 0), stop=(dc == DC - 1),
                            )
                        nc.scalar.add(qt[:, hp, :], psq[:, 0:QC], bq[:, hp:hp + 1])

                    # attnT[p, dc, j] = attn^T[dc*128+p, qc*QC+j]
                    attnT = p2a.tile([P, DC, QC], f32r, tag="attnT")
                    for hg in range(NHG):
                        heads = range(hg * HG, (hg + 1) * HG)
                        pvs = {h: pvps.tile([PD + 1, QC], f32, tag=f"pv{h % HG}",
                                            name=f"pv{h}")
                               for h in heads}
                        nkc = (qc + 1) * NKT // NQC if mode == "tril" else NKT
                        for kc in range(nkc):
                            if mode == "add":
                                mt = p2s.tile([P, QC], f32, tag="mt")
                            elif mode in ("bin", "tril"):
                                mt = p2s.tile([P, QC], bf16, tag="mt")
                            if mode != "none":
                                nc.sync.dma_start(
                                    mt[:],
                                    maskT_d[kc * P:(kc + 1) * P,
                                            qc * QC:(qc + 1) * QC],
                                )
                            for h in heads:
                                po = (h % 2) * PD
                                pss = qkps.tile([P, D], f32, tag="qk")
                                nc.tensor.matmul(
                                    pss[:, 0:QC],
                                    kts[kc // (SC // P)][
                                        po:po + PD, h // 2,
                                        (kc % (SC // P)) * P:
                                        (kc % (SC // P) + 1) * P],
                                    qt[po:po + PD, h // 2, :],
                                    start=True, stop=True,
                                )
                                pt = p2s.tile([P, QC], bf16, tag="pt")
                                if mode == "add":
                                    st = p2s.tile([P, QC], f32, tag="st")
                                    nc.vector.scalar_tensor_tensor(
                                        out=st[:], in0=mt[:], scalar=-1e9,
                                        in1=pss[:, 0:QC],
                                        op0=Alu.mult, op1=Alu.add,
                                    )
                                    nc.scalar.activation(pt[:], st[:], Exp,
                                                         scale=0.125)
                                elif mode in ("bin", "tril"):
                                    pr = p2s.tile([P, QC], bf16, tag="pr")
                                    nc.scalar.activation(pr[:], pss[:, 0:QC], Exp,
                                                         scale=0.125)
                                    nc.vector.tensor_mul(
                                        out=pt[:], in0=pr[:], in1=mt[:])
                                else:
                                    nc.scalar.activation(pt[:], pss[:, 0:QC], Exp,
                                                         scale=0.125)
                                nc.tensor.matmul(
                                    pvs[h][:],
                                    vts[kc][:, h, :],
                                    pt[:],
                                    start=(kc == 0), stop=(kc == nkc - 1),
                                    skip_group_check=True,
                                )
                        for h in heads:
                            recip = p2s.tile([1, QC], f32r, tag="recip")
                            nc.vector.reciprocal(recip[:], pvs[h][PD:PD + 1, :])
                            bcp = fps.tile([PD, QC], f32, tag="fin")
                            nc.tensor.matmul(
                                bcp[:], ones_col[:], recip[:],
                                start=True, stop=True,
                            )
                            bcs = p2s.tile([PD, QC], f32, tag="bcs")
                            nc.vector.tensor_copy(out=bcs[:], in_=bcp[:])
                            po = (h % 2) * PD
                            nc.vector.tensor_mul(
                                out=attnT[po:po + PD, h // 2, :],
                                in0=pvs[h][0:PD, :],
                                in1=bcs[:],
                            )

                    # output projection for this q-chunk
                    for rt in range(QC // P):
                        psf = fps.tile([P, D], f32, tag="fin")
                        for dc in range(DC):
                            nc.tensor.matmul(
                                psf[:],
                                attnT[:, dc, rt * P:(rt + 1) * P],
                                wo[:, dc, :],
                                start=(dc == 0), stop=(dc == DC - 1),
                            )
                        osb = p2s.tile([P, D], f32, tag="osb")
                        nc.vector.tensor_add(out=osb[:], in0=psf[:], in1=bo[:])
                        nc.sync.dma_start(
                            out_d[qc * QC + rt * P: qc * QC + (rt + 1) * P, :],
                            osb[:],
                        )
    nc.finalize()
    return nc


def _get_prog(mode: str):
    if mode not in _prog_cache:
        _prog_cache[mode] = _build(mode)
    return _prog_cache[mode]


def _q_rows(c, mode):
    """Query rows (into this core's batch) owned by core c."""
    if mode == "tril":
        # interleaved 128-row blocks so the causal kv range per q-chunk is
        # identical on every core
        j = np.arange(QR // P)
        base = (j * CPB + (c % CPB)) * P
        return (base[:, None] + np.arange(P)[None, :]).ravel()
    r0 = (c % CPB) * QR
    return np.arange(r0, r0 + QR)


def _prep_shared(Wq, bq, Wk, bk, Wv, bv, Wo, bo):
    def warr(W):
        return np.ascontiguousarray(
            np.asarray(W, dtype=np.float32).reshape(DC, P, D).transpose(1, 0, 2))

    def barr(b):
        return np.ascontiguousarray(
            np.asarray(b, dtype=np.float32).reshape(DC, P).T)

    return {
        "wq": warr(Wq), "wk": warr(Wk), "wv": warr(Wv),
        "wo": warr(Wo),
        "bq": barr(bq), "bk": barr(bk),
        "bv": np.ascontiguousarray(
            np.broadcast_to(np.asarray(bv, dtype=np.float32), (P, D))),
        "bo": np.ascontiguousarray(
            np.broadcast_to(np.asarray(bo, dtype=np.float32), (P, D))),
    }


def make_in_maps(inputs, mask, Wq, bq, Wk, bk, Wv, bv, Wo, bo):
    import ml_dtypes
    inputs = np.asarray(inputs, dtype=np.float32)
    mask = np.asarray(mask, dtype=np.float32)
    if not np.any(mask):
        mode = "none"
        maskT = None
    elif np.array_equal(mask, np.triu(np.ones((S, S), dtype=np.float32), 1)):
        mode = "tril"
        maskT = np.ascontiguousarray(1.0 - mask.T).astype(ml_dtypes.bfloat16)
    elif bool(((mask == 0.0) | (mask == 1.0)).all()):
        mode = "bin"
        maskT = np.ascontiguousarray(1.0 - mask.T).astype(ml_dtypes.bfloat16)
    else:
        mode = "add"
        maskT = np.ascontiguousarray(mask.T)
    shared = _prep_shared(Wq, bq, Wk, bk, Wv, bv, Wo, bo)

    in_maps = []
    for c in range(NCORES):
        b = c // CPB
        rows = _q_rows(c, mode)
        m = dict(shared)
        m["ones"] = np.ones((P, H), dtype=ml_dtypes.bfloat16)
        m["onesr"] = np.ones((1, PD), dtype=np.float32)
        m["xb"] = np.ascontiguousarray(inputs[b])
        m["xq"] = np.ascontiguousarray(inputs[b][rows])
        if maskT is not None:
            m["maskT"] = np.ascontiguousarray(maskT[:, rows])
        in_maps.append(m)
    return mode, in_maps


def assemble(results, mode):
    out = np.empty((B, S, D), dtype=np.float32)
    for c in range(NCORES):
        b = c // CPB
        out[b, _q_rows(c, mode)] = results[c]["out"]
    return out


def kernel(inputs, mask, Wq, bq, Wk, bk, Wv, bv, Wo, bo):
    from concourse.bass_utils import run_bass_kernel_spmd

    mode, in_maps = make_in_maps(inputs, mask, Wq, bq, Wk, bk, Wv, bv, Wo, bo)
    nc = _get_prog(mode)
    res = run_bass_kernel_spmd(nc, in_maps, core_ids=list(range(NCORES)))
    return assemble(res.results, mode)


# revision 26
# speedup vs baseline: 2.0220x; 1.0094x over previous
"""Multi-head attention (B=2, S=4096, D=512, H=8) on 8 trn2 NeuronCores.

Sharding: query-row data-parallel. Core c handles batch c//4, query rows
(c%4)*1024:(c%4+1)*1024 — all 8 heads. Each core:
  phase 1: PE-transposes its batch's X, projects K^T [D,S] and V [S,D]
           (full sequence, replicated across the 4 cores of a batch).
  phase 2: per 256-wide query chunk: project Q^T, then flash-style
           attention fully on-chip in transposed layout:
           S^T[k,q] = (K^T slice).T @ (Q^T slice) on PE, optional additive
           mask on DVE, exp on ACT, P^T @ V_aug on PE where V_aug carries
           a ones column so the softmax denominator falls out of the same
           matmul; per-head normalization via reciprocal + rank-1 PE
           broadcast.
  phase 3: output projection with per-head [64,q] lhsT chunks + bias.
No collectives; the host slices inputs per core and concatenates outputs.
Matmuls run as float32r (full-rate fp32) with fp32 PSUM accumulation.
"""

import numpy as np

# Problem dims (hardcoded per contract)
B, S, D, H, PD = 2, 4096, 512, 8, 64
P = 128
NCORES = 8
CPB = 4           # cores per batch
QR = S // CPB     # 1024 query rows per core
QC = 512          # query-chunk width in attention phase
NQC = QR // QC    # 2
SC = 512          # sequence chunk in KV-projection phase
NSC = S // SC     # 8
NKT = S // P      # 32 key tiles of 128
DC = D // P       # 4 d-chunks of 128
HP = H // 2       # 4 head-pairs
HG = 4            # heads per attention group (PSUM budget)
NHG = H // HG     # 2

_prog_cache = {}


def _build(mode: str):
    import concourse.mybir as mybir
    import concourse.tile as tile
    from concourse import bacc
    from concourse.masks import make_identity

    f32 = mybir.dt.float32
    f32r = mybir.dt.float32r
    bf16 = mybir.dt.bfloat16
    Exp = mybir.ActivationFunctionType.Exp
    Alu = mybir.AluOpType

    nc = bacc.Bacc(debug=False, target_bir_lowering=False)

    xb = nc.declare_dram_parameter("xb", [S, D], f32, isOutput=False)
    xq = nc.declare_dram_parameter("xq", [QR, D], f32, isOutput=False)
    wq_d = nc.declare_dram_parameter("wq", [P, DC, D], bf16, isOutput=False)
    wk_d = nc.declare_dram_parameter("wk", [P, DC, D], bf16, isOutput=False)
    wv_d = nc.declare_dram_parameter("wv", [P, DC, D], bf16, isOutput=False)
    wo_d = nc.declare_dram_parameter("wo", [P, DC, D], f32r, isOutput=False)
    bq_d = nc.declare_dram_parameter("bq", [P, DC], f32, isOutput=False)
    bk_d = nc.declare_dram_parameter("bk", [P, DC], f32, isOutput=False)
    bv_d = nc.declare_dram_parameter("bv", [P, D], f32, isOutput=False)
    bo_d = nc.declare_dram_parameter("bo", [P, D], f32, isOutput=False)
    ones_d = nc.declare_dram_parameter("ones", [P, H], bf16, isOutput=False)
    onesr_d = nc.declare_dram_parameter("onesr", [1, PD], f32r, isOutput=False)
    if mode == "add":
        maskT_d = nc.declare_dram_parameter("maskT", [S, QR], f32, isOutput=False)
    elif mode in ("bin", "tril"):
        maskT_d = nc.declare_dram_parameter("maskT", [S, QR], bf16, isOutput=False)
    out_d = nc.declare_dram_parameter("out", [QR, D], f32, isOutput=True)

    with tile.TileContext(nc) as tc, nc.allow_low_precision(
            reason="float32r tiles are 4-byte fp32; PE rounds reads only"):
        with (
            tc.tile_pool(name="const", bufs=1) as constp,
            tc.tile_pool(name="kt", bufs=1) as ktp,
            tc.tile_pool(name="vt", bufs=1) as vtp,
            tc.tile_pool(name="work", bufs=2) as work,
        ):
            ident = constp.tile([P, P], f32, tag="ident")
            make_identity(nc, ident)
            ones_col = constp.tile([1, PD], f32r, tag="ones")
            nc.sync.dma_start(ones_col[:], onesr_d[:])

            # weights/biases needed beyond phase 1
            wq = constp.tile([P, DC, D], bf16, tag="wq")
            wo = constp.tile([P, DC, D], f32r, tag="wo")
            bq = constp.tile([P, DC], f32, tag="bq")
            bo = constp.tile([P, D], f32, tag="bo")
            for sb_t, dr_t in [(wq, wq_d), (wo, wo_d), (bq, bq_d), (bo, bo_d)]:
                nc.sync.dma_start(sb_t[:], dr_t[:])

            # K^T tiles: kts[sci][p, hp, j] = K^T[hp*128+p, sci*SC+j]
            kts = [ktp.tile([P, HP, SC], bf16, tag=f"kt{i}", name=f"kt{i}")
                   for i in range(NSC)]
            # V tiles: vts[kti][p, h, 0:64] = V[kti*128+p, h*64:(h+1)*64],
            # col 64 = 1.0 (softmax-denominator row of the PV matmul)
            vts = [vtp.tile([P, H, PD + 1], bf16, tag=f"v{i}", name=f"v{i}")
                   for i in range(NKT)]
            for t in vts:
                nc.sync.dma_start(t[:, :, PD:PD + 1], ones_d[:, :, None])

            # ---- phase 1: transpose X chunks, project K^T and V ----
            with (
                tc.tile_pool(name="p1w", bufs=1) as p1w,
                tc.tile_pool(name="ps1", bufs=2, space="PSUM") as ps1,
            ):
                wk = p1w.tile([P, DC, D], bf16, tag="wk")
                wv = p1w.tile([P, DC, D], bf16, tag="wv")
                bk = p1w.tile([P, DC], f32, tag="bk")
                bv = p1w.tile([P, D], f32, tag="bv")
                for sb_t, dr_t in [(wk, wk_d), (wv, wv_d), (bk, bk_d), (bv, bv_d)]:
                    nc.sync.dma_start(sb_t[:], dr_t[:])

                for sci in range(NSC):
                    xraw = work.tile([P, SC // P, D], f32, tag="xraw")
                    nc.sync.dma_start(
                        xraw[:],
                        xb[sci * SC:(sci + 1) * SC].rearrange(
                            "(rt p) d -> p rt d", p=P),
                    )
                    xt = work.tile([P, DC, SC], bf16, tag="xt")
                    for rt in range(SC // P):
                        pst = ps1.tile([P, D], f32, tag="tps")
                        for dc in range(DC):
                            nc.tensor.transpose(
                                pst[:, dc * P:(dc + 1) * P],
                                xraw[:, rt, dc * P:(dc + 1) * P],
                                ident[:],
                            )
                        # pst[p, dc*128+j] = X^T[dc*128+p, sci*SC+rt*128+j]
                        nc.scalar.copy(
                            out=xt[:, :, rt * P:(rt + 1) * P],
                            in_=pst[:].rearrange("p (dc j) -> p dc j", dc=DC),
                        )
                    # K^T for this chunk
                    for hp in range(HP):
                        psk = ps1.tile([P, SC], f32, tag="kproj")
                        for dc in range(DC):
                            nc.tensor.matmul(
                                psk[:],
                                wk[:, dc, hp * P:(hp + 1) * P],
                                xt[:, dc, :],
                                start=(dc == 0), stop=(dc == DC - 1),
                            )
                        nc.scalar.add(kts[sci][:, hp, :], psk[:], bk[:, hp:hp + 1])
                    # V for this chunk
                    for rt in range(SC // P):
                        psv = ps1.tile([P, D], f32, tag="vproj")
                        for dc in range(DC):
                            nc.tensor.matmul(
                                psv[:],
                                xt[:, dc, rt * P:(rt + 1) * P],
                                wv[:, dc, :],
                                start=(dc == 0), stop=(dc == DC - 1),
                            )
                        kti = sci * (SC // P) + rt
                        nc.vector.tensor_add(
                            out=vts[kti][:, :, 0:PD],
                            in0=psv[:].rearrange("p (h d) -> p h d", h=H),
                            in1=bv[:].rearrange("p (h d) -> p h d", h=H),
                        )

            # ---- phase 2+3: attention + output projection per q-chunk ----
            with (
                tc.tile_pool(name="p2", bufs=2) as p2,
                tc.tile_pool(name="p2s", bufs=3) as p2s,
                tc.tile_pool(name="p2a", bufs=1) as p2a,
                tc.tile_pool(name="qkps", bufs=3, space="PSUM") as qkps,
                tc.tile_pool(name="pvps", bufs=1, space="PSUM") as pvps,
                tc.tile_pool(name="fps", bufs=1, space="PSUM") as fps,
            ):
                for qc in range(NQC):
                    # Q^T for this chunk: qt[p, hp, j] = Q^T[hp*128+p, qc*QC+j]
                    xqraw = work.tile([P, QC // P, D], f32, tag="xraw")
                    nc.sync.dma_start(
                        xqraw[:],
                        xq[qc * QC:(qc + 1) * QC].rearrange(
                            "(rt p) d -> p rt d", p=P),
                    )
                    xqt = work.tile([P, DC, QC], bf16, tag="xt")
                    for rt in range(QC // P):
                        pst = qkps.tile([P, D], f32, tag="qk")
                        for dc in range(DC):
                            nc.tensor.transpose(
                                pst[:, dc * P:(dc + 1) * P],
                                xqraw[:, rt, dc * P:(dc + 1) * P],
                                ident[:],
                            )
                        nc.scalar.copy(
                            out=xqt[:, :, rt * P:(rt + 1) * P],
                            in_=pst[:].rearrange("p (dc j) -> p dc j", dc=DC),
                        )
                    qt = p2.tile([P, HP, QC], bf16, tag="qt")
                    for hp in range(HP):
                        psq = qkps.tile([P, D], f32, tag="qk")
                        for dc in range(DC):
                            nc.tensor.matmul(
                                psq[:, 0:QC],
                                wq[:, dc, hp * P:(hp + 1) * P],
                                xqt[:, dc, :],
                                start=(dc == 0), stop=(dc == DC - 1),
                            )
                        nc.scalar.add(qt[:, hp, :], psq[:, 0:QC], bq[:, hp:hp + 1])

                    # attnT[p, dc, j] = attn^T[dc*128+p, qc*QC+j]
                    attnT = p2a.tile([P, DC, QC], f32r, tag="attnT")
                    for hg in range(NHG):
                        heads = range(hg * HG, (hg + 1) * HG)
                        pvs = {h: pvps.tile([PD + 1, QC], f32, tag=f"pv{h % HG}",
                                            name=f"pv{h}")
                               for h in heads}
                        nkc = (qc + 1) * NKT // NQC if mode == "tril" else NKT
                        for kc in range(nkc):
                            if mode == "add":
                                mt = p2s.tile([P, QC], f32, tag="mt")
                            elif mode in ("bin", "tril"):
                                mt = p2s.tile([P, QC], bf16, tag="mt")
                            if mode != "none":
                                nc.sync.dma_start(
                                    mt[:],
                                    maskT_d[kc * P:(kc + 1) * P,
                                            qc * QC:(qc + 1) * QC],
                                )
                            for h in heads:
                                po = (h % 2) * PD
                                pss = qkps.tile([P, D], f32, tag="qk")
                                nc.tensor.matmul(
                                    pss[:, 0:QC],
                                    kts[kc // (SC // P)][
                                        po:po + PD, h // 2,
                                        (kc % (SC // P)) * P:
                                        (kc % (SC // P) + 1) * P],
                                    qt[po:po + PD, h // 2, :],
                                    start=True, stop=True,
                                )
                                pt = p2s.tile([P, QC], bf16, tag="pt")
                                if mode == "add":
                                    st = p2s.tile([P, QC], f32, tag="st")
                                    nc.vector.scalar_tensor_tensor(
                                        out=st[:], in0=mt[:], scalar=-1e9,
                                        in1=pss[:, 0:QC],
                                        op0=Alu.mult, op1=Alu.add,
                                    )
                                    nc.scalar.activation(pt[:], st[:], Exp,
                                                         scale=0.125)
                                elif mode in ("bin", "tril"):
                                    pr = p2s.tile([P, QC], bf16, tag="pr")
                                    nc.scalar.activation(pr[:], pss[:, 0:QC], Exp,
                                                         scale=0.125)
                                    nc.vector.tensor_mul(
                                        out=pt[:], in0=pr[:], in1=mt[:])
                                else:
                                    nc.scalar.activation(pt[:], pss[:, 0:QC], Exp,
                                                         scale=0.125)
                                nc.tensor.matmul(
                                    pvs[h][:],
                                    vts[kc][:, h, :],
                                    pt[:],
                                    start=(kc == 0), stop=(kc == nkc - 1),
                                    skip_group_check=True,
                                )
                        for h in heads:
                            recip = p2s.tile([1, QC], f32r, tag="recip")
                            nc.vector.reciprocal(recip[:], pvs[h][PD:PD + 1, :])
                            bcp = fps.tile([PD, QC], f32, tag="fin")
                            nc.tensor.matmul(
                                bcp[:], ones_col[:], recip[:],
                                start=True, stop=True,
                            )
                            bcs = p2s.tile([PD, QC], f32, tag="bcs")
                            nc.vector.tensor_copy(out=bcs[:], in_=bcp[:])
                            po = (h % 2) * PD
                            nc.vector.tensor_mul(
                                out=attnT[po:po + PD, h // 2, :],
                                in0=pvs[h][0:PD, :],
                                in1=bcs[:],
                            )

                    # output projection for this q-chunk
                    for rt in range(QC // P):
                        psf = fps.tile([P, D], f32, tag="fin")
                        for dc in range(DC):
                            nc.tensor.matmul(
                                psf[:],
                                attnT[:, dc, rt * P:(rt + 1) * P],
                                wo[:, dc, :],
                                start=(dc == 0), stop=(dc == DC - 1),
                            )
                        osb = p2s.tile([P, D], f32, tag="osb")
                        nc.vector.tensor_add(out=osb[:], in0=psf[:], in1=bo[:])
                        nc.sync.dma_start(
                            out_d[qc * QC + rt * P: qc * QC + (rt + 1) * P, :],
                            osb[:],
                        )
    nc.finalize()
    return nc


def _get_prog(mode: str):
    if mode not in _prog_cache:
        _prog_cache[mode] = _build(mode)
    return _prog_cache[mode]


def _q_rows(c, mode):
    """Query rows (into this core's batch) owned by core c."""
    if mode == "tril":
        # interleaved 128-row blocks so the causal kv range per q-chunk is
        # identical on every core
        j = np.arange(QR // P)
        base = (j * CPB + (c % CPB)) * P
        return (base[:, None] + np.arange(P)[None, :]).ravel()
    r0 = (c % CPB) * QR
    return np.arange(r0, r0 + QR)


def _prep_shared(Wq, bq, Wk, bk, Wv, bv, Wo, bo):
    import ml_dtypes

    def warr(W):
        return np.ascontiguousarray(
            np.asarray(W, dtype=np.float32).reshape(DC, P, D)
            .transpose(1, 0, 2)).astype(ml_dtypes.bfloat16)

    def barr(b):
        return np.ascontiguousarray(
            np.asarray(b, dtype=np.float32).reshape(DC, P).T)

    return {
        "wq": warr(Wq), "wk": warr(Wk), "wv": warr(Wv),
        "wo": np.ascontiguousarray(
            np.asarray(Wo, dtype=np.float32).reshape(DC, P, D)
            .transpose(1, 0, 2)),
        "bq": barr(bq), "bk": barr(bk),
        "bv": np.ascontiguousarray(
            np.broadcast_to(np.asarray(bv, dtype=np.float32), (P, D))),
        "bo": np.ascontiguousarray(
            np.broadcast_to(np.asarray(bo, dtype=np.float32), (P, D))),
    }


def make_in_maps(inputs, mask, Wq, bq, Wk, bk, Wv, bv, Wo, bo):
    import ml_dtypes
    inputs = np.asarray(inputs, dtype=np.float32)
    mask = np.asarray(mask, dtype=np.float32)
    if not np.any(mask):
        mode = "none"
        maskT = None
    elif np.array_equal(mask, np.triu(np.ones((S, S), dtype=np.float32), 1)):
        mode = "tril"
        maskT = np.ascontiguousarray(1.0 - mask.T).astype(ml_dtypes.bfloat16)
    elif bool(((mask == 0.0) | (mask == 1.0)).all()):
        mode = "bin"
        maskT = np.ascontiguousarray(1.0 - mask.T).astype(ml_dtypes.bfloat16)
    else:
        mode = "add"
        maskT = np.ascontiguousarray(mask.T)
    shared = _prep_shared(Wq, bq, Wk, bk, Wv, bv, Wo, bo)

    in_maps = []
    for c in range(NCORES):
        b = c // CPB
        rows = _q_rows(c, mode)
        m = dict(shared)
        m["ones"] = np.ones((P, H), dtype=ml_dtypes.bfloat16)
        m["onesr"] = np.ones((1, PD), dtype=np.float32)
        m["xb"] = np.ascontiguousarray(inputs[b])
        m["xq"] = np.ascontiguousarray(inputs[b][rows])
        if maskT is not None:
            m["maskT"] = np.ascontiguousarray(maskT[:, rows])
        in_maps.append(m)
    return mode, in_maps


def assemble(results, mode):
    out = np.empty((B, S, D), dtype=np.float32)
    for c in range(NCORES):
        b = c // CPB
        out[b, _q_rows(c, mode)] = results[c]["out"]
    return out


def kernel(inputs, mask, Wq, bq, Wk, bk, Wv, bv, Wo, bo):
    from concourse.bass_utils import run_bass_kernel_spmd

    mode, in_maps = make_in_maps(inputs, mask, Wq, bq, Wk, bk, Wv, bv, Wo, bo)
    nc = _get_prog(mode)
    res = run_bass_kernel_spmd(nc, in_maps, core_ids=list(range(NCORES)))
    return assemble(res.results, mode)
